# revision 94
# baseline (speedup 1.0000x reference)
"""Trainium2 Bass kernel v3 for nn_Based_40630390620259 (sparse_attention).

v3 keeps the baseline's numeric profile (bf16 data paths, fp8 only on the
q/k score matmuls — value-path fp8 fails the 2e-2 gate because diffuse
attention over zero-mean values passes relative noise through undamped)
but restructures the dataflow:

1. Zero-padded DoubleRow subtiles kill ALL partition-repack DMAs: q/k score
   operands live at their natural projection partitions with a zeroed
   second subtile (win qw8n/kw8n [128, 2, S]; lin qkg8n [64, 2qk, 2sub, S]
   with a constant ones slot at (p0,sub1)/(p32,sub1) folding the
   feature-map +1, so no affine is needed before the Square).
2. The projection writes lin q and k to separate PSUM partition halves
   (padded col layout) so the on-chip quantize is a partition-shifted copy.
3. Input DMAs are ordered so the first projection starts ~1.5us in; the
   32 repack DMAs and their Pool-engine SWDGE overhead (~24us) are gone.
4. Lin squares split ACT (head a, direct PSUM Square) / DVE (head b,
   staging copy + multiply); masks stay on Pool.

Sharding: tensor-parallel over heads, 2 heads per core, 8 cores; each core
produces a partial [S, D] bf16 output (host sums in f32).
"""

import numpy as np
import ml_dtypes

S = 2048
D = 1024
H = 16
FD = 16
HD = 64
W = 256
NCORES = 8
NT = 16          # 128-row t/s chunks
SB = 512         # s block width (4 chunks)
NJ = 4
NRL = 6          # lin strip ring depth
NRW = 6          # win strip ring depth

BF = ml_dtypes.bfloat16
E4 = ml_dtypes.float8_e4m3fn

_CACHE = {}


def _build_nc(lin_dr=True, dbg=False):
    import concourse.bass as bass
    import concourse.mybir as mybir
    import concourse.tile as tile
    from concourse import bacc
    from concourse.bass import ts

    f32 = mybir.dt.float32
    bf16 = mybir.dt.bfloat16
    fp8 = mybir.dt.float8e4
    MULT = mybir.AluOpType.mult
    DR = mybir.MatmulPerfMode.DoubleRow
    Exp = mybir.ActivationFunctionType.Exp
    Square = mybir.ActivationFunctionType.Square
    Copy = mybir.ActivationFunctionType.Copy

    nc = bacc.Bacc("TRN2", target_bir_lowering=False)

    ht_d = nc.dram_tensor("ht", [128, 8, S], bf16, kind="ExternalInput")
    # wqk cols: 0:64 winq-a | 64:128 winq-b | 128:192 wink-a | 192:256
    # wink-b | 256:320 lin-q [qa16 z16 qb16 z16] | 320:384 lin-k [ka z kb z]
    wqk_d = nc.dram_tensor("wqk", [128, 8, 384], bf16, kind="ExternalInput")
    wv_d = nc.dram_tensor("wv", [128, 8, 256], bf16, kind="ExternalInput")
    wo_d = nc.dram_tensor("wo", [256, D], bf16, kind="ExternalInput")
    msk_d = nc.dram_tensor("msk", [128, 256], bf16, kind="ExternalInput")
    tril_d = nc.dram_tensor("tril", [128, 128], bf16, kind="ExternalInput")
    ones_d = nc.dram_tensor("ones", [128, 128], bf16, kind="ExternalInput")
    oneh_d = nc.dram_tensor("oneh", [128, NT * 16], bf16,
                            kind="ExternalInput")
    bsel_d = nc.dram_tensor("bsel", [16, NT * 128], bf16,
                            kind="ExternalInput")
    lin1_d = nc.dram_tensor("lin1", [64, 2, S], fp8, kind="ExternalInput")
    zs_d = nc.dram_tensor("zs", [128, 2048], fp8, kind="ExternalInput")
    out_d = nc.dram_tensor("out", [S, D], bf16, kind="ExternalOutput")

    def dma(out, in_, q=0):
        eng = (nc.sync, nc.scalar, nc.gpsimd)[q]
        eng.dma_start(out=out, in_=in_)

    def dma_t(out, in_):
        nc.sync.dma_start_transpose(out=out, in_=in_)

    with tile.TileContext(nc) as tc:
        with (
            tc.tile_pool(name="sb", bufs=1) as sb,
            tc.tile_pool(name="mcp", bufs=2) as mcp,
            tc.tile_pool(name="psA", bufs=1, space="PSUM") as psA,
            tc.tile_pool(name="psB", bufs=1, space="PSUM") as psB,
        ):
            # ---------------- persistent SBUF ----------------
            ht_sb = sb.tile([128, 8, S], bf16, name="ht_sb")
            wqk_sb = sb.tile([128, 8, 384], bf16, name="wqk_sb")
            wv_sb = sb.tile([128, 8, 256], bf16, name="wv_sb")
            wo_sb = sb.tile([128, 2, D], bf16, name="wo_sb")
            msk_sb = sb.tile([128, 256], bf16, name="msk_sb")
            tril_sb = sb.tile([128, 128], bf16, name="tril_sb")
            ones_sb = sb.tile([128, 128], bf16, name="ones_sb")
            oneh_sb = sb.tile([128, NT * 16], bf16, name="oneh_sb")
            bsel_sb = sb.tile([16, NT * 128], bf16, name="bsel_sb")

            qw8n = sb.tile([128, 2, S], fp8, name="qw8n")
            kw8n = sb.tile([128, 2, S], fp8, name="kw8n")
            qkg8n = sb.tile([64, 2, 2, S], fp8, name="qkg8n")

            v4_sb = sb.tile([128, NT, 260], bf16, name="v4_sb")
            csum_sb = sb.tile([16, 130], bf16, name="csum_sb")
            U_sb = sb.tile([128, NJ, 1040], f32, name="U_sb")
            rec_sb = sb.tile([128, NJ, 16], f32, name="rec_sb")
            scl_sb = sb.tile([128, NT, 256], bf16, name="scl_sb")
            sclT = sb.tile([128, 2, NT, 128], bf16, name="sclT")

            # 16 ti-slots: each j's accs are emitted before the next j's
            # strips reuse the slots (deps serialize the reuse)
            lring = sb.tile([128, 22, 2, SB], bf16, name="lring")
            wring = sb.tile([128, 22, 2, SB], bf16, name="wring")
            so_j = sb.tile([128, 4, 2, SB], bf16, name="so_j")

            # ---------------- PSUM ----------------
            accP = [psB.tile([128, 512], f32, name=f"accP{i}") for i in range(3)]
            poP = psB.tile([128, 512], f32, name="poP")

            def acc_slot(G):
                b, o = (G // 7, G % 7) if G < 14 else (2, G - 14)
                return accP[b], o * 65

            opened = set()

            def bank_mm(G, lhsT, rhs, stop):
                bank, off = acc_slot(G)
                bid = 0 if G < 7 else (1 if G < 14 else 2)
                st = bid not in opened
                opened.add(bid)
                nc.tensor.matmul(
                    bank[:, off : off + 65],
                    lhsT=lhsT,
                    rhs=rhs,
                    start=st,
                    stop=stop,
                    skip_group_check=True,
                )

            # ---------------- input loads ----------------
            # critical path: wqk blk0 + first ht quarter feed the first proj
            wqr = wqk_d.rearrange("p k (b x) -> p k b x", x=128)
            wqs = wqk_sb.rearrange("p k (b x) -> p k b x", x=128)
            dma(wqs[:, :, 0, :], wqr[:, :, 0, :], q=0)
            # first quarter in fine slices so the first proj pieces can
            # start as soon as each lands
            dma(ht_sb[:, :, ts(0, 128)], ht_d[:, :, ts(0, 128)], q=1)
            dma(wqs[:, :, 1, :], wqr[:, :, 1, :], q=0)
            dma(ht_sb[:, :, 128:256], ht_d[:, :, 128:256], q=1)
            dma(wqs[:, :, 2, :], wqr[:, :, 2, :], q=0)
            dma(ht_sb[:, :, 256:512], ht_d[:, :, 256:512], q=1)
            dma(wv_sb[:, :, :], wv_d[:, :, :], q=0)
            dma(ht_sb[:, :, ts(1, 512)], ht_d[:, :, ts(1, 512)], q=1)
            dma(ht_sb[:, :, ts(2, 512)], ht_d[:, :, ts(2, 512)], q=0)
            dma(ht_sb[:, :, ts(3, 512)], ht_d[:, :, ts(3, 512)], q=1)
            # constants / zero-fills (off the critical path)
            dma(qw8n[:, 1, :], zs_d[:, :], q=2)
            dma(kw8n[:, 1, :], zs_d[:, :], q=2)
            dma(qkg8n[:, :, 1, :], lin1_d[:, :, :], q=2)
            dma(wo_sb[:, :, :], wo_d.rearrange("(k p) x -> p k x", p=128), q=2)
            dma(msk_sb[:, :], msk_d[:, :], q=2)
            dma(tril_sb[:, :], tril_d[:, :], q=2)
            dma(ones_sb[:, :], ones_d[:, :], q=2)
            dma(oneh_sb[:, :], oneh_d[:, :], q=2)
            dma(bsel_sb[:, :], bsel_d[:, :], q=2)

            v4g = v4_sb.rearrange("p c (g x) -> p c g x", x=65)
            nc.gpsimd.memset(v4g[:, :, :, 64], 0.5)

            # ---------------- phase 1: projections ----------------
            def proj(jp, jh, blk, ptile, pieces=((0, 512),), grp=(True, True),
                     quantize=True):
                c0 = blk * 128
                jq = 2 * jp + jh
                for pi, (cs, ce) in enumerate(pieces):
                    for k in range(8):
                        nc.tensor.matmul(
                            ptile[:, cs:ce],
                            lhsT=wqk_sb[:, k, c0 : c0 + 128],
                            rhs=ht_sb[:, k, jq * 512 + cs : jq * 512 + ce],
                            start=(k == 0 and pi == 0 and grp[0]),
                            stop=(k == 7 and pi == len(pieces) - 1 and grp[1]),
                            skip_group_check=True,
                        )
                if not quantize:
                    return
                jsh = ts(jq, SB)
                if blk == 0:
                    nc.vector.tensor_scalar(
                        out=qw8n[:, 0, jsh], in0=ptile[:, :],
                        scalar1=8.0, scalar2=None, op0=MULT,
                    )
                elif blk == 1:
                    nc.scalar.activation(
                        kw8n[:, 0, jsh], ptile[:, :], Copy, scale=8.0
                    )
                else:
                    nc.scalar.activation(
                        qkg8n[:, 0, 0, jsh], ptile[0:64, :], Copy, scale=0.5
                    )
                    nc.scalar.activation(
                        qkg8n[:, 1, 0, jsh], ptile[64:128, :], Copy, scale=0.5
                    )

            def vproj(sp_, pv):
                st0 = 2 * sp_
                for sh in range(2):
                    for k in range(8):
                        nc.tensor.matmul(
                            pv[:, sh * 256 : sh * 256 + 256],
                            lhsT=ht_sb[:, k, ts(st0 + sh, 128)],
                            rhs=wv_sb[:, k, :],
                            start=(k == 0 and sh == 0),
                            stop=(k == 7),
                            skip_group_check=True,
                        )
                pvr = pv[:, 0:512].rearrange("p (s g x) -> p s g x", s=2, x=64)
                nc.vector.tensor_copy(
                    out=v4g[:, st0 : st0 + 2, :, 0:64], in_=pvr
                )

            def chunksums():
                for c in range(NT):
                    nc.tensor.matmul(
                        poP[0:16, 0:130],
                        lhsT=oneh_sb[:, ts(c, 16)],
                        rhs=v4_sb[:, c, 0:130],
                        start=(c == 0),
                        stop=(c == NT - 1),
                        skip_group_check=True,
                    )
                nc.vector.tensor_copy(out=csum_sb[:, :], in_=poP[0:16, 0:130])

            # phase 1 rotates through the psB banks while they are free
            # (quarters 0-1); once j0's accs claim them it falls back to
            # poP-only with strip/acc work filling the drain bubbles
            p1_rr = {"i": 0, "pop_only": False}

            def ptile_next():
                if p1_rr["pop_only"]:
                    return poP
                t = (poP, accP[0], accP[1], accP[2])[p1_rr["i"] % 4]
                p1_rr["i"] += 1
                return t

            first_pieces = ((0, 128), (128, 256), (256, 512))
            prefill = []

            # ---------------- phase 2 ----------------
            eng_rr = {"i": 0}

            def rr_copy(out, in_, tail=False):
                eng_rr["i"] += 1
                if tail and eng_rr["i"] % 2 == 0:
                    nc.scalar.copy(out=out, in_=in_)
                else:
                    nc.vector.tensor_copy(out=out, in_=in_)

            fillers = prefill

            def slot(j, ti):
                return 16 + ti if j == 0 else ti

            def lim_l(j):
                return 4 * j + 4

            def lim_w(j):
                return min(NT, 4 * j + 6)

            def lin_strip(j, ti):
                js0 = j * SB
                col0 = max(0, ti - 4 * j) * 128
                r = slot(j, ti)

                # lin uses the OPPOSITE bank pair from win of the same ti:
                # consecutive strips of each kind alternate pairs (depth 2)
                spp = psA.tile([128, 2, SB], f32, name="spp",
                               tag=("B", "A")[ti % 2])
                mpa = spp[:, 0, :]
                mpb = spp[:, 1, :]
                mph = (mpa, mpb)
                pieces = [(c, min(c + 256, SB)) for c in range(col0, SB, 256)]
                for h in range(2):
                    for cs, ce in pieces:
                        nc.tensor.matmul(
                            mph[h][:, cs:ce],
                            lhsT=qkg8n[32 * h : 32 * h + 16, 1, :, ts(ti, 128)],
                            rhs=qkg8n[32 * h : 32 * h + 16, 0, :,
                                      js0 + cs : js0 + ce],
                            start=True,
                            stop=True,
                            perf_mode=DR,
                        )

                def sq_act(h):
                    nc.scalar.activation(
                        lring[:, r, h, col0:SB], mph[h][:, col0:SB], Square,
                        scale=1.0,
                    )

                def sq_dve(h):
                    mc = mcp.tile([128, SB], bf16, name="mc", tag="mc")
                    nc.vector.tensor_scalar(
                        out=mc[:, col0:SB], in0=mph[h][:, col0:SB],
                        scalar1=1.0, scalar2=None, op0=MULT,
                    )
                    nc.vector.tensor_tensor(
                        out=lring[:, r, h, col0:SB], in0=mc[:, col0:SB],
                        in1=mc[:, col0:SB], op=MULT,
                    )

                # alternate the square engine so neither ACT nor DVE
                # rate-limits the strip loop
                if ti % 2 == 0:
                    sq_act(0)
                    sq_dve(1)
                else:
                    sq_dve(0)
                    sq_act(1)
                sd = ti - 4 * j
                if 0 <= sd <= 3:
                    for h in range(2):
                        nc.gpsimd.tensor_tensor(
                            out=lring[:, r, h, ts(sd, 128)],
                            in0=lring[:, r, h, ts(sd, 128)],
                            in1=msk_sb[:, 0:128],
                            op=MULT,
                        )

            def win_strip(j, ti):
                js0 = j * SB
                col0 = max(0, ti - 2 - 4 * j) * 128
                r = slot(j, ti)
                spr = psA.tile([128, 2, SB], f32, name="wsp",
                               tag=("A", "B")[ti % 2])
                pieces = [(c, min(c + 256, SB)) for c in range(col0, SB, 256)]
                for h in range(2):
                    for cs, ce in pieces:
                        nc.tensor.matmul(
                            spr[:, h, cs:ce],
                            lhsT=kw8n[64 * h : 64 * h + 64, :, ts(ti, 128)],
                            rhs=qw8n[64 * h : 64 * h + 64, :,
                                     js0 + cs : js0 + ce],
                            start=True,
                            stop=True,
                            perf_mode=DR,
                        )
                nc.scalar.activation(
                    wring[:, r, :, col0:SB], spr[:, :, col0:SB], Exp,
                    scale=1.0 / 512.0,
                )
                sd = ti - 2 - 4 * j
                if 0 <= sd <= 3:
                    for h in range(2):
                        nc.gpsimd.tensor_tensor(
                            out=wring[:, r, h, ts(sd, 128)],
                            in0=wring[:, r, h, ts(sd, 128)],
                            in1=msk_sb[:, 128:256],
                            op=MULT,
                        )

            def acc_mm(ring, r, ti, scl_, g, stop):
                gi = g % 2
                bank_mm(4 * scl_ + g,
                        ring[:, r, gi, ts(scl_, 128)],
                        v4_sb[:, ti, 65 * g : 65 * g + 65], stop)

            def cum_tril(j, scl_, h):
                sc = 4 * j + scl_
                bank_mm(4 * scl_ + h, tril_sb[:, :],
                        v4_sb[:, sc, 65 * h : 65 * h + 65], True)

            def cum_base(j, scl_, h):
                sc = 4 * j + scl_
                bank_mm(4 * scl_ + h, bsel_sb[:, ts(sc, 128)],
                        csum_sb[:, 65 * h : 65 * h + 65], False)

            def cum_base0(scl_, h):
                for c in range(scl_):
                    bank_mm(4 * scl_ + h, ones_sb[:, :],
                            v4_sb[:, c, 65 * h : 65 * h + 65], False)

            def bank_copy(j, b):
                lo = [0, 455, 910][b]
                wdt = [455, 455, 130][b]
                if j == NJ - 1 and b == 1:
                    nc.scalar.copy(
                        out=U_sb[:, j, lo : lo + wdt], in_=accP[b][:, 0:wdt]
                    )
                else:
                    nc.vector.tensor_copy(
                        out=U_sb[:, j, lo : lo + wdt], in_=accP[b][:, 0:wdt]
                    )

            def retire_pieces(j, scl_):
                sc = 4 * j + scl_
                pieces = []

                def recip():
                    uj = U_sb.rearrange("p j (G x) -> p j G x", x=65)
                    nc.vector.reciprocal(
                        out=rec_sb[:, j, 4 * scl_ : 4 * scl_ + 4],
                        in_=uj[:, j, 4 * scl_ : 4 * scl_ + 4, 64],
                    )

                pieces.append(recip)

                def div(g):
                    uj = U_sb.rearrange("p j (G x) -> p j G x", x=65)
                    nc.gpsimd.tensor_scalar(
                        out=scl_sb[:, sc, 64 * g : 64 * g + 64],
                        in0=uj[:, j, 4 * scl_ + g, 0:64],
                        scalar1=rec_sb[:, j, 4 * scl_ + g : 4 * scl_ + g + 1],
                        scalar2=None,
                        op0=MULT,
                    )

                def transp(cg):
                    dma_t(sclT[:, cg, sc, :], scl_sb[:, sc, ts(cg, 128)])

                # transpose cg0 right after its two divs so the final matmul
                # chain starts earlier
                pieces.append(lambda: div(0))
                pieces.append(lambda: div(1))
                pieces.append(lambda: transp(0))
                pieces.append(lambda: div(2))
                pieces.append(lambda: div(3))
                pieces.append(lambda: transp(1))

                def final(nb):
                    if j == NJ - 1:
                        # strips are done: alternate with a freed psA slot so
                        # the copy of final k overlaps the matmul of k+1
                        fp = psA.tile([128, 2, SB], f32, name="fin",
                                      tag=("A", "B")[(2 * scl_ + nb) % 2]
                                      )[:, 0, :]
                    else:
                        fp = poP
                    for cg in range(2):
                        nc.tensor.matmul(
                            fp[:, :],
                            lhsT=sclT[:, cg, sc, :],
                            rhs=wo_sb[:, cg, ts(nb, SB)],
                            start=(cg == 0),
                            stop=(cg == 1),
                            skip_group_check=True,
                        )
                    rr_copy(so_j[:, scl_, nb, :], fp[:, :],
                            tail=(j == NJ - 1))

                def store_sc():
                    dma(out_d[ts(sc, 128), :],
                        so_j[:, scl_, :, :].rearrange("p n x -> p (n x)"),
                        q=0)

                pieces.append(lambda: final(0))
                pieces.append(lambda: final(1))
                pieces.append(store_sc)
                return pieces

            def pump(n):
                for _ in range(n):
                    if fillers:
                        fillers.pop(0)()

            LAG = 3

            def emit_accs(j, ti):
                ll, lw = lim_l(j), lim_w(j)
                if ti < ll:
                    rl = slot(j, ti)
                    for scl_ in range(4):
                        sc = 4 * j + scl_
                        if j > 0 and ti == 0 and sc > 0:
                            cum_base(j, scl_, 0)
                            cum_base(j, scl_, 1)
                        if ti < sc:
                            for h in range(2):
                                acc_mm(lring, rl, ti, scl_, h, stop=False)
                        elif ti == sc:
                            if j == 0 and scl_ > 0:
                                cum_base0(scl_, 0)
                                cum_base0(scl_, 1)
                            for h in range(2):
                                acc_mm(lring, rl, ti, scl_, h, stop=False)
                            cum_tril(j, scl_, 0)
                            cum_tril(j, scl_, 1)
                rw = slot(j, ti)
                for scl_ in range(4):
                    sc = 4 * j + scl_
                    if ti <= min(sc + 2, lw - 1):
                        sp_ = (ti == min(sc + 2, lw - 1))
                        for h in range(2):
                            acc_mm(wring, rw, ti, scl_, 2 + h, stop=sp_)

            def retire_j(j):
                for b in range(3):
                    bank_copy(j, b)
                chunk_pieces = [retire_pieces(j, scl_) for scl_ in range(4)]
                for scl_ in range(4):
                    for pi in range(7):
                        fillers.append(chunk_pieces[scl_][pi])
                if j == NJ - 1:
                    # split the last stores per chunk so they overlap the
                    # remaining finals instead of trailing them
                    for pi in (7, 8):
                        for scl_ in range(4):
                            fillers.append(chunk_pieces[scl_][pi])
                            if pi == 8:
                                fillers.append(chunk_pieces[scl_][9])
                else:
                    for pi in (7, 8):
                        for scl_ in range(4):
                            fillers.append(chunk_pieces[scl_][pi])

                    def store_j():
                        dma(
                            out_d.rearrange("(b p) d -> p b d", p=128)[
                                :, 4 * j : 4 * j + 4, :
                            ],
                            so_j.rearrange("p s n x -> p s (n x)"),
                            q=0,
                        )

                    fillers.append(store_j)

            # ---------------- global schedule ----------------
            # quarters of phase 1 feed strips as soon as quantized q/k
            # exist; accs of block j interleave with strips of block j+1
            # (freeing ring slots just before reuse); retirement pieces pump
            # into PE drain bubbles
            # quarter 0
            ptiles = [ptile_next() for _ in range(3)]
            for pi, (cs, ce) in enumerate(first_pieces):
                last = pi == len(first_pieces) - 1
                for blk in range(3):
                    proj(0, 0, blk, ptiles[blk], pieces=((cs, ce),),
                         grp=(pi == 0, last), quantize=last)
            vproj(0, ptile_next())
            vproj(1, ptile_next())
            # quarter 1
            for blk in range(3):
                proj(0, 1, blk, ptile_next())
            for ti in range(6):
                win_strip(0, ti)
                if ti < 4:
                    lin_strip(0, ti)
            vproj(2, ptile_next())
            vproj(3, ptile_next())
            for ti in range(8):
                win_strip(1, ti)
                lin_strip(1, ti)
            # quarter 2
            for blk in range(3):
                proj(1, 0, blk, ptile_next())
            win_strip(1, 8)
            win_strip(1, 9)
            vproj(4, ptile_next())
            vproj(5, ptile_next())
            # quarter 3
            for blk in range(3):
                proj(1, 1, blk, ptile_next())
            vproj(6, ptile_next())
            vproj(7, ptile_next())
            chunksums()
            for j in range(NJ):
                opened.clear()
                ll, lw = lim_l(j), lim_w(j)
                for ti in range(lw):
                    if ti >= LAG:
                        emit_accs(j, ti - LAG)
                    if j > 1:
                        if ti < ll:
                            lin_strip(j, ti)
                        pump(1)
                        win_strip(j, ti)
                    pump(2)
                for ti in range(lw - LAG, lw):
                    emit_accs(j, ti)
                    pump(1)
                retire_j(j)
            pump(len(fillers))

    nc.compile()
    return nc


def _prep_inputs(inputs):
    h = np.asarray(inputs["hidden_states"], np.float32).reshape(S, D)
    ht = np.ascontiguousarray(h.T)  # [D, S]
    ht_p = np.ascontiguousarray(
        ht.reshape(8, 128, S).transpose(1, 0, 2)).astype(BF)

    lin_Wq = np.asarray(inputs["lin_Wq"], np.float32)
    lin_Wk = np.asarray(inputs["lin_Wk"], np.float32)
    lin_Wv = np.asarray(inputs["lin_Wv"], np.float32)
    lin_Wo = np.asarray(inputs["lin_Wo"], np.float32)
    win_Wq = np.asarray(inputs["win_Wq"], np.float32)
    win_Wk = np.asarray(inputs["win_Wk"], np.float32)
    win_Wv = np.asarray(inputs["win_Wv"], np.float32)
    win_Wo = np.asarray(inputs["win_Wo"], np.float32)

    p = np.arange(128)[:, None]
    f = np.arange(128)[None, :]
    msk = np.zeros((128, 256), np.float32)
    msk[:, 0:128] = (p <= f)
    msk[:, 128:256] = (p < f)
    tril = (p <= f).astype(np.float32)
    ones = np.ones((128, 128), np.float32)
    oneh = np.zeros((128, NT * 16), np.float32)
    for c in range(NT):
        oneh[:, c * 16 + c] = 1.0
    bsel = np.zeros((16, NT * 128), np.float32)
    for sc in range(NT):
        bsel[:sc, sc * 128 : (sc + 1) * 128] = 1.0
    lin1 = np.zeros((64, 2, S), np.float32)
    lin1[0, :, :] = 1.0
    lin1[32, :, :] = 1.0
    zs = np.zeros((128, 2048), np.float32)

    in_maps = []
    for c in range(NCORES):
        a, b = 2 * c, 2 * c + 1
        wqk = np.zeros((D, 384), np.float32)
        wqk[:, 0:64] = win_Wq[:, a * HD : (a + 1) * HD]
        wqk[:, 64:128] = win_Wq[:, b * HD : (b + 1) * HD]
        wqk[:, 128:192] = win_Wk[:, a * HD : (a + 1) * HD]
        wqk[:, 192:256] = win_Wk[:, b * HD : (b + 1) * HD]
        wqk[:, 256:272] = lin_Wq[:, a * FD : (a + 1) * FD]
        wqk[:, 288:304] = lin_Wq[:, b * FD : (b + 1) * FD]
        wqk[:, 320:336] = lin_Wk[:, a * FD : (a + 1) * FD]
        wqk[:, 352:368] = lin_Wk[:, b * FD : (b + 1) * FD]
        wqk_p = np.ascontiguousarray(
            wqk.reshape(8, 128, 384).transpose(1, 0, 2)).astype(BF)
        wv = np.zeros((D, 256), np.float32)
        wv[:, 0:64] = lin_Wv[:, a * HD : (a + 1) * HD]
        wv[:, 64:128] = lin_Wv[:, b * HD : (b + 1) * HD]
        wv[:, 128:192] = win_Wv[:, a * HD : (a + 1) * HD]
        wv[:, 192:256] = win_Wv[:, b * HD : (b + 1) * HD]
        wv_p = np.ascontiguousarray(
            (0.5 * wv).reshape(8, 128, 256).transpose(1, 0, 2)).astype(BF)
        wo = np.zeros((256, D), np.float32)
        wo[0:64] = lin_Wo[a * HD : (a + 1) * HD]
        wo[64:128] = lin_Wo[b * HD : (b + 1) * HD]
        wo[128:192] = win_Wo[a * HD : (a + 1) * HD]
        wo[192:256] = win_Wo[b * HD : (b + 1) * HD]
        in_maps.append(
            {
                "ht": ht_p,
                "wqk": wqk_p,
                "wv": wv_p,
                "wo": wo.astype(BF),
                "msk": msk.astype(BF),
                "tril": tril.astype(BF),
                "ones": ones.astype(BF),
                "oneh": oneh.astype(BF),
                "bsel": bsel.astype(BF),
                "lin1": lin1.astype(E4),
                "zs": zs.astype(E4),
            }
        )
    return in_maps


def kernel(**inputs) -> np.ndarray:
    from concourse.bass_utils import run_bass_kernel_spmd

    if "nc" not in _CACHE:
        _CACHE["nc"] = _build_nc()
    nc = _CACHE["nc"]
    in_maps = _prep_inputs(inputs)
    res = run_bass_kernel_spmd(nc, in_maps, core_ids=list(range(NCORES)))
    out = np.zeros((S, D), np.float32)
    for r in res.results:
        out += np.asarray(r["out"], np.float32)
    return out.reshape(1, S, D)


if __name__ == "__main__":
    nc = _build_nc()
    print("built ok")
